# revision 46
# baseline (speedup 1.0000x reference)
"""Trainium2 Bass kernel for Conv2dWeightModulate (StyleGAN2-style modulated conv).

Math restructure 1 (modulation): the per-sample modulated conv
    out[b] = conv(conv_w * c * style[b,cin] * sigma_inv[b,cout], x_pad[b])
is rewritten as
    out[b,cout] = sigma_inv[b,cout] * conv(conv_w, (x[b] * c*style[b,cin])_pad)
so the conv weights are sample-independent (resident in SBUF) and the
per-sample modulation becomes a per-input-channel scale of x plus a
per-output-channel scale of the result. sigma has the closed form
    sigma^2[b,cout] = c^2 * sum_cin style[b,cin]^2 * sum_k conv_w[cout,cin,k]^2
computed on host (tiny [B,CIN] x [CIN,COUT] product), as is the 3-layer
mapping network producing style (all [16,512]-sized, <0.01% of FLOPs).

Math restructure 2 (Winograd F(4,3) along the height axis): each quad of
output rows (4q..4q+3) is computed from 6 transformed input rows
    v0 = 4d0-5d2+d4, v1 = -4(d1+d2)+(d3+d4), v2 = 4(d1-d2)+(d4-d3),
    v3 = 2(d3-d1)+(d4-d2), v4 = -2(d3-d1)+(d4-d2), v5 = 4d1-5d3+d5
with host-transformed weights U = G @ w over the kh axis
    (G = [[1/4,0,0],[-1/6,-1/6,-1/6],[-1/6,1/6,-1/6],
          [1/24,1/12,1/6],[1/24,-1/12,1/6],[0,0,1]])
and output rows
    out0 = m0+m1+m2+m3+m4,  out1 = m1-m2+2(m3-m4),
    out2 = (m1+m2)+4(m3+m4), out3 = m1-m2+8(m3-m4)+m5
where M[pos] = sum_{cin,kw} U[pos,kw] * v[pos] (shifted by kw).
This cuts tensor-engine MACs 2x vs direct conv (18 accumulation steps per
8 output rows instead of 36); the width axis stays direct (3 taps against
a replicate-padded 66-wide SBUF image).

Device: data-parallel over batch, 2 samples per core on 8 cores, fp16
operands (PE at 1 col/cycle, fp32 PSUM accumulate). Per (sample, chunk of
8 quads, cout-block): 6 PSUM banks hold M[0..5] for 8 quads x 64 cols; 72
accumulating 128x128 @ 128x512 matmuls fill them pos-major; VectorE forms
the four output-row combinations in fp32 (using fused scalar_tensor_tensor
for the x2/x4/x8 taps), ScalarE applies sigma_inv, and the input row
transform for the NEXT chunk is interleaved into the vector queue between
combine groups so the tensor engine never waits on it. A per-sample
power-of-2 prescale keeps scaled x in fp16's normal range and is undone
exactly in the fp32 output scale.
"""

import numpy as np
from contextlib import ExitStack

import concourse.bass as bass
import concourse.tile as tile
from concourse import bacc, mybir
from concourse import bass_utils

B, CIN, COUT, KS, H, W, DLAT = 16, 512, 512, 3, 64, 64, 512
EPS = 1e-8
N_CORES = 8
SPC = B // N_CORES          # samples per core
NCB = CIN // 128            # cin blocks
NOB = COUT // 128           # cout blocks
NPOS = 6                    # winograd F(4,3) positions
NQ = H // 4                 # row quads per sample (16)
QPC = 8                     # quads per PSUM chunk (8 quads * 64 = 512)
NCH = NQ // QPC             # chunks per sample (2)
PADW = W + 2
NROWS = 33                  # original rows staged per chunk
_cache = {}

_MUL = mybir.AluOpType.mult
_ADD = mybir.AluOpType.add


def _build():
    if "nc" in _cache:
        return _cache["nc"]
    f32 = mybir.dt.float32
    f16 = mybir.dt.float16
    nc = bacc.Bacc("TRN2", target_bir_lowering=False, debug=False,
                   num_devices=N_CORES)
    x_d = nc.dram_tensor("x", [SPC, CIN, H, W], f16, kind="ExternalInput").ap()
    # U[pos, ob, cb, cin128, kw*cout128]
    wt_d = nc.dram_tensor("wt", [NPOS, NOB, NCB, 128, KS * 128], f16,
                          kind="ExternalInput").ap()
    sig_d = nc.dram_tensor("sig", [128, SPC, NOB], f32,
                           kind="ExternalInput").ap()
    out_d = nc.dram_tensor("out", [SPC, COUT, H * W], f32,
                           kind="ExternalOutput").ap()

    with tile.TileContext(nc) as tc, ExitStack() as ctx:
        cpool = ctx.enter_context(tc.tile_pool(name="const", bufs=1))
        stpool = ctx.enter_context(tc.tile_pool(name="stage", bufs=4))
        vpool = ctx.enter_context(tc.tile_pool(name="v", bufs=2))
        vtpool = ctx.enter_context(tc.tile_pool(name="vt", bufs=8))
        ctpool = ctx.enter_context(tc.tile_pool(name="ct", bufs=8))
        opool = ctx.enter_context(tc.tile_pool(name="o", bufs=3))
        epool = ctx.enter_context(tc.tile_pool(name="e", bufs=8))
        pspool = ctx.enter_context(tc.tile_pool(name="ps", bufs=8, space="PSUM"))

        wt_sb = cpool.tile([128, NPOS, NOB, NCB, KS, 128], f16)
        sig_sb = cpool.tile([128, SPC, NOB], f32)

        # PE pre-warm: dummy matmuls bridging until the first real matmul
        # (~13us in), so the HAM clock-gate stays at 8/8 throughout
        warm_t = cpool.tile([128, QPC * W], f16)
        warm_ps = pspool.tile([128, QPC * W], f32, name="warm_ps", tag="ps")
        nc.gpsimd.memset(warm_t[:], 0.0)
        for _ in range(80):
            nc.tensor.matmul(warm_ps[:, 0:64], warm_t[:, 0:128],
                             warm_t[:, 0:64], start=True, stop=True)
        for _ in range(24):
            nc.tensor.matmul(warm_ps[:], warm_t[:, 0:128], warm_t[:],
                             start=True, stop=True)

        st_tiles = {}

        def emit_st_dmas(s, ch, cbs=range(NCB)):
            # stage the 33 original rows covering chunk ch of sample s
            r0 = 0 if ch == 0 else H - NROWS
            for cb in cbs:
                st = stpool.tile([128, NROWS * W], f16, name="st", tag="st")
                nc.sync.dma_start(
                    st[:],
                    x_d[s, cb * 128:(cb + 1) * 128,
                        r0:r0 + NROWS, :].rearrange("c a b -> c (a b)"))
                st_tiles[(s, ch, cb)] = st

        def v_tile(s, ch):
            key = ("v", s, ch)
            if key not in _cache_v:
                _cache_v[key] = vpool.tile([128, NCB, NPOS, QPC, PADW], f16,
                                           name=f"v{s}{ch}", tag="v")
            return _cache_v[key]

        _cache_v = {}

        def emit_vbuild_pos(s, ch, cb, tag):
            """Build V positions for one cin block: tag in {0,1,2,34,5}.

            d_i views: chunk0 uses d_i = st[4q+i-1] (pad row 0 duplicated,
            so only v0's q=0 needs a fixup); chunk1 uses d_i = st[4q+i]
            (pad row 65 duplicated, so only v5's q=7 needs a fixup).
            Replicate-padding commutes with the (linear) row transform, so
            V's column pads are plain copies. pos3/4 and the pads run on
            the otherwise-idle GpSimd engine to unload VectorE.
            """
            v_t = v_tile(s, ch)
            off = -1 if ch == 0 else 0
            st = st_tiles[(s, ch, cb)]
            sr = st.rearrange("c (a b) -> c a b", b=W)

            def d(i, q0=0, nq=QPC):
                lo = 4 * q0 + i + off
                return sr[:, lo:lo + 4 * (nq - 1) + 1:4]

            vi = v_t[:, cb]
            if tag == 0:
                # pos0: v0 = 4 d0 - 5 d2 + d4
                g1 = vtpool.tile([128, QPC, W], f16, name="g1", tag="vt")
                nc.vector.scalar_tensor_tensor(
                    g1[:], d(2), -5.0, d(4), _MUL, _ADD)
                if ch == 0:
                    nc.vector.scalar_tensor_tensor(
                        vi[:, 0, 1:, 1:W + 1], d(0, q0=1, nq=QPC - 1),
                        4.0, g1[:, 1:], _MUL, _ADD)
                    # q=0 fixup: d0 = clamped row = sr[0]
                    nc.vector.scalar_tensor_tensor(
                        vi[:, 0, 0:1, 1:W + 1], sr[:, 0:1], 4.0,
                        g1[:, 0:1], _MUL, _ADD)
                else:
                    nc.vector.scalar_tensor_tensor(
                        vi[:, 0, :, 1:W + 1], d(0), 4.0, g1[:], _MUL, _ADD)
            elif tag == 1:
                # pos1: v1 = -4(d1+d2) + (d3+d4)
                s12 = vtpool.tile([128, QPC, W], f16, name="s12", tag="vt")
                t34 = vtpool.tile([128, QPC, W], f16, name="t34", tag="vt")
                nc.vector.tensor_add(s12[:], d(1), d(2))
                nc.vector.tensor_add(t34[:], d(3), d(4))
                nc.vector.scalar_tensor_tensor(
                    vi[:, 1, :, 1:W + 1], s12[:], -4.0, t34[:], _MUL, _ADD)
            elif tag == 2:
                # pos2: v2 = 4(d1-d2) + (d4-d3)
                m12 = vtpool.tile([128, QPC, W], f16, name="m12", tag="vt")
                n43 = vtpool.tile([128, QPC, W], f16, name="n43", tag="vt")
                nc.vector.tensor_sub(m12[:], d(1), d(2))
                nc.vector.tensor_sub(n43[:], d(4), d(3))
                nc.vector.scalar_tensor_tensor(
                    vi[:, 2, :, 1:W + 1], m12[:], 4.0, n43[:], _MUL, _ADD)
            elif tag == 34:
                # pos3: v3 = 2(d3-d1) + (d4-d2); pos4: v4 = -2(d3-d1)+(d4-d2)
                a = vtpool.tile([128, QPC, W], f16, name="a", tag="vt")
                bb = vtpool.tile([128, QPC, W], f16, name="b", tag="vt")
                nc.vector.tensor_sub(a[:], d(3), d(1))
                nc.vector.tensor_sub(bb[:], d(4), d(2))
                nc.vector.scalar_tensor_tensor(
                    vi[:, 3, :, 1:W + 1], a[:], 2.0, bb[:], _MUL, _ADD)
                nc.vector.scalar_tensor_tensor(
                    vi[:, 4, :, 1:W + 1], a[:], -2.0, bb[:], _MUL, _ADD)
            elif tag == -34:
                # tag 34 on GpSimd (TT-only: no scalar_tensor_tensor there);
                # used for the latency-critical first chunk to parallelize
                # the transform across engines
                a = vtpool.tile([128, QPC, W], f16, name="a", tag="vt")
                bb = vtpool.tile([128, QPC, W], f16, name="b", tag="vt")
                a2 = vtpool.tile([128, QPC, W], f16, name="a2", tag="vt")
                nc.gpsimd.tensor_sub(a[:], d(3), d(1))
                nc.gpsimd.tensor_sub(bb[:], d(4), d(2))
                nc.gpsimd.tensor_add(a2[:], a[:], a[:])
                nc.gpsimd.tensor_add(vi[:, 3, :, 1:W + 1], a2[:], bb[:])
                nc.gpsimd.tensor_sub(vi[:, 4, :, 1:W + 1], bb[:], a2[:])
            elif tag == 5:
                # pos5: v5 = 4 d1 - 5 d3 + d5
                g2 = vtpool.tile([128, QPC, W], f16, name="g2", tag="vt")
                if ch == 0:
                    nc.vector.scalar_tensor_tensor(
                        g2[:], d(3), -5.0, d(5), _MUL, _ADD)
                    nc.vector.scalar_tensor_tensor(
                        vi[:, 5, :, 1:W + 1], d(1), 4.0, g2[:], _MUL, _ADD)
                else:
                    nc.vector.scalar_tensor_tensor(
                        g2[:, 0:QPC - 1], d(3, nq=QPC - 1), -5.0,
                        d(5, nq=QPC - 1), _MUL, _ADD)
                    # q=7 fixup: d5 = clamped row = sr[32]
                    nc.vector.scalar_tensor_tensor(
                        g2[:, QPC - 1:QPC], sr[:, 31:32], -5.0,
                        sr[:, 32:33], _MUL, _ADD)
                    nc.vector.scalar_tensor_tensor(
                        vi[:, 5, :, 1:W + 1], d(1), 4.0, g2[:], _MUL, _ADD)

        def emit_pads(s, ch, sl):
            # V column pads (replicate: padded col0==col1, 65==64) for the
            # given position slice, across all cin blocks, on ScalarE
            v_t = v_tile(s, ch)
            nc.scalar.copy(v_t[:, :, sl, :, 0], v_t[:, :, sl, :, 1])
            nc.scalar.copy(v_t[:, :, sl, :, W + 1], v_t[:, :, sl, :, W])

        def emit_vbuild_group(s, ch, group):
            # group 0: pos 0,1; group 1: pos 2,3,4; group 2: pos 5
            tags = ((0, 1), (2, 34), (5,))[group]
            for cb in range(NCB):
                for t in tags:
                    emit_vbuild_pos(s, ch, cb, t)
            emit_pads(s, ch, (slice(0, 2), slice(2, 5), slice(5, 6))[group])

        def emit_tileset(s, ch, ob, last=False, split_dma=False):
            v_t = v_tile(s, ch)
            pts = [pspool.tile([128, QPC * W], f32, name="ps", tag="ps")
                   for _ in range(NPOS)]
            es = [epool.tile([128, QPC * W], f32, name=f"e{i}", tag="e")
                  for i in range(NPOS)]
            ct = {}

            def combine(nm, fn):
                t = ctpool.tile([128, QPC * W], f32, name=nm, tag="ct")
                fn(t)
                ct[nm] = t

            # row-phase i of quad p is output row 4p+i: storing [p][i][w]
            # makes both sides of the output DMA fully contiguous
            out4 = opool.tile([128, QPC, 4, W], f32, name="out4", tag="o")
            outs = [out4[:, :, i, :] for i in range(4)]
            row0 = ch * QPC * 4
            dst = out_d[s, ob * 128:(ob + 1) * 128,
                        row0 * W:(row0 + 4 * QPC) * W]

            def mm(pos, cb):
                for kw in range(KS):
                    nc.tensor.matmul(
                        pts[pos][:],
                        wt_sb[:, pos, ob, cb, kw, :],
                        v_t[:, cb, pos, :, kw:kw + W],
                        start=(cb == 0 and kw == 0),
                        stop=(cb == NCB - 1 and kw == KS - 1))

            def post(pos):
                # sigma_inv-scaled PSUM eviction on ScalarE right after the
                # position's accumulation stops (frees the bank early); the
                # A^T combines then run purely in SBUF on VectorE
                nc.scalar.mul(es[pos][:], pts[pos][:], sig_sb[:, s, ob:ob + 1])
                if pos == 2:
                    # p=m1+m2, q=m1-m2, u=m0+p (all pre-scaled by sigma);
                    # in the final chunk p/q go to GpSimd so the vector
                    # queue drains before the last matmuls finish
                    eng = nc.gpsimd if last else nc.vector
                    combine("p", lambda t: eng.tensor_add(
                        t[:], es[1][:], es[2][:]))
                    combine("q", lambda t: eng.tensor_sub(
                        t[:], es[1][:], es[2][:]))
                    combine("u", lambda t: nc.vector.tensor_add(
                        t[:], ct["p"][:], es[0][:]))
                elif pos == 4:
                    # r=m3+m4, t=m3-m4, then out0..2 and the out3 partial
                    combine("r", lambda t: nc.vector.tensor_add(
                        t[:], es[3][:], es[4][:]))
                    combine("t", lambda t: nc.vector.tensor_sub(
                        t[:], es[3][:], es[4][:]))
                    r3 = lambda t: t.rearrange("c (p w) -> c p w", w=W)
                    nc.vector.tensor_add(outs[0], r3(ct["u"][:]),
                                         r3(ct["r"][:]))
                    nc.vector.scalar_tensor_tensor(
                        outs[1], r3(ct["t"][:]), 2.0, r3(ct["q"][:]),
                        _MUL, _ADD)
                    nc.vector.scalar_tensor_tensor(
                        outs[2], r3(ct["r"][:]), 4.0, r3(ct["p"][:]),
                        _MUL, _ADD)
                    combine("s3", lambda t: nc.vector.scalar_tensor_tensor(
                        t[:], ct["t"][:], 8.0, ct["q"][:], _MUL, _ADD))
                    if split_dma:
                        # drain row phases 0-2 early so the final chunk's
                        # tail is not gated on one big transfer
                        d4 = dst.rearrange("c (p i w) -> c p i w", i=4, w=W)
                        for i in range(3):
                            nc.sync.dma_start(d4[:, :, i, :], outs[i])
                elif pos == 5:
                    r3 = lambda t: t.rearrange("c (p w) -> c p w", w=W)
                    nc.vector.tensor_add(outs[3], r3(ct["s3"][:]),
                                         r3(es[5][:]))
                    if split_dma:
                        d4 = dst.rearrange("c (p i w) -> c p i w", i=4, w=W)
                        nc.sync.dma_start(d4[:, :, 3, :], outs[3])
                    else:
                        # one merged, fully-contiguous DMA for all 32 rows
                        nc.sync.dma_start(
                            dst, out4.rearrange("c p i w -> c (p i w)"))

            for pos in range(NPOS):
                for cb in range(NCB):
                    mm(pos, cb)
                post(pos)

        # ob-0 weight slices land pos-major first (~2.4MB) so the first
        # tile-set never waits on the 9.4MB full weight transfer; the
        # remaining ob slices follow in bulk (one DMA per pos each way to
        # keep the Sync issue queue short)
        def emit_wt_ob0(pos):
            nc.sync.dma_start(
                wt_sb[:, pos, 0].rearrange("c b k w -> c b (k w)"),
                wt_d[pos, 0].rearrange("b c k -> c b k"))

        def emit_wt_rest(pos):
            nc.sync.dma_start(
                wt_sb[:, pos, 1:].rearrange("c o b k w -> c o b (k w)"),
                wt_d[pos, 1:].rearrange("o b c k -> c o b k"))

        chunks = [(s, ch) for s in range(SPC) for ch in range(NCH)]
        # DMA order: x block 0 and the pos-0/ob-0 weights first, so block
        # 0's transform and its pos-0 matmuls start as early as possible
        emit_st_dmas(*chunks[0], cbs=(0,))
        emit_wt_ob0(0)
        emit_st_dmas(*chunks[0], cbs=(1, 2, 3))
        nc.sync.dma_start(sig_sb[:], sig_d[:])
        for pos in range(1, NPOS):
            emit_wt_ob0(pos)
        # first chunk: pos-granular transform emission with per-cb pads so
        # the pos-0 matmuls of each cin block start as soon as that block
        # is built (matmul waits are per-instruction in the tensor queue);
        # pos 3/4 of blocks 2-3 run on GpSimd to halve the serial latency
        s0, c0 = chunks[0]
        v_t0 = v_tile(s0, c0)
        for ti, tag in enumerate((0, 1, 2, 34, 5)):
            sl = (slice(0, 1), slice(1, 2), slice(2, 3),
                  slice(3, 5), slice(5, 6))[ti]
            for cb in range(NCB):
                emit_vbuild_pos(s0, c0, cb,
                                -34 if (tag == 34 and cb >= 2) else tag)
                nc.scalar.copy(v_t0[:, cb, sl, :, 0], v_t0[:, cb, sl, :, 1])
                nc.scalar.copy(v_t0[:, cb, sl, :, W + 1],
                               v_t0[:, cb, sl, :, W])
            if ti < 3:
                emit_wt_rest(2 * ti)
                emit_wt_rest(2 * ti + 1)
        for ci, (s, ch) in enumerate(chunks):
            for ob in range(NOB):
                tl = ci == len(chunks) - 1
                emit_tileset(s, ch, ob, last=tl,
                             split_dma=(tl and ob == NOB - 1))
                # interleave the next chunk's input transform into the
                # vector queue so it overlaps this chunk's matmuls
                if ci + 1 < len(chunks) and ob < 3:
                    if ob == 0:
                        emit_st_dmas(*chunks[ci + 1])
                    emit_vbuild_group(*chunks[ci + 1], ob)
    nc.compile()
    _cache["nc"] = nc
    return nc


def _prelu(z, a):
    return np.where(z >= 0, z, a * z)


_G = np.array([[1 / 4, 0, 0],
               [-1 / 6, -1 / 6, -1 / 6],
               [-1 / 6, 1 / 6, -1 / 6],
               [1 / 24, 1 / 12, 1 / 6],
               [1 / 24, -1 / 12, 1 / 6],
               [0, 0, 1]], dtype=np.float64)


def _prepare(inputs):
    x = np.asarray(inputs["x"], dtype=np.float32)
    s = np.asarray(inputs["s"], dtype=np.float32)
    map_w0 = np.asarray(inputs["map_w0"], dtype=np.float32)
    map_b0 = np.asarray(inputs["map_b0"], dtype=np.float32)
    a0 = np.asarray(inputs["prelu_a0"], dtype=np.float32)
    map_w1 = np.asarray(inputs["map_w1"], dtype=np.float32)
    map_b1 = np.asarray(inputs["map_b1"], dtype=np.float32)
    a1 = np.asarray(inputs["prelu_a1"], dtype=np.float32)
    style_w = np.asarray(inputs["style_w"], dtype=np.float32)
    style_b = np.asarray(inputs["style_b"], dtype=np.float32)
    conv_w = np.asarray(inputs["conv_w"], dtype=np.float32)

    c_lin = np.float32(1.0 / np.sqrt(DLAT))
    z = _prelu(s @ (map_w0 * c_lin).T + map_b0, a0)
    z = _prelu(z @ (map_w1 * c_lin).T + map_b1, a1)
    style = z @ (style_w * c_lin).T + style_b          # [B, CIN]

    c_conv = 1.0 / np.sqrt(CIN * KS * KS)
    w2 = ((conv_w.astype(np.float64) * c_conv) ** 2).sum(axis=(2, 3))  # [COUT, CIN]
    sig2 = (style.astype(np.float64) ** 2) @ w2.T                      # [B, COUT]
    sig_inv = (1.0 / np.sqrt(sig2 + EPS)).astype(np.float32)
    msc = (style * np.float32(c_conv)).astype(np.float32)              # [B, CIN]

    # per-sample power-of-2 normalizer keeps msc*x in fp16's normal range;
    # undone exactly in the fp32 output scale
    rms = np.sqrt(np.mean((msc.astype(np.float64)) ** 2, axis=1)) + 1e-30
    k = np.clip(np.round(-np.log2(rms)), -20, 40).astype(np.int32)     # [B]
    pw = np.exp2(k.astype(np.float32))                                  # 2^k
    msc_n = msc * pw[:, None]
    sig_n = sig_inv / pw[:, None]

    # fold the per-cin style scale into x on host, cast fp16
    x_scaled = (x * msc_n[:, :, None, None]).astype(np.float16)

    # winograd F(4,3) weight transform over kh: U[pos] = sum_kh G[pos,kh] w
    # conv_w: [COUT, CIN, KH, KW] -> U: [NPOS, NOB, NCB, 128cin, KW*128cout]
    u = np.einsum("pk,ockw->ocpw", _G, conv_w.astype(np.float64)).astype(np.float16)
    wt_host = np.ascontiguousarray(
        u.reshape(NOB, 128, NCB, 128, NPOS, KS).transpose(4, 0, 2, 3, 5, 1)
        .reshape(NPOS, NOB, NCB, 128, KS * 128))

    sig_r = sig_n.reshape(B, NOB, 128)
    in_maps = []
    for c in range(N_CORES):
        sl = slice(c * SPC, (c + 1) * SPC)
        in_maps.append({
            "x": np.ascontiguousarray(x_scaled[sl]),
            "wt": wt_host,
            "sig": np.ascontiguousarray(sig_r[sl].transpose(2, 0, 1)),
        })
    return in_maps


def run(inputs, **spmd_kwargs):
    nc = _build()
    in_maps = _prepare(inputs)
    res = bass_utils.run_bass_kernel_spmd(
        nc, in_maps, core_ids=list(range(N_CORES)), **spmd_kwargs)
    out = np.concatenate(
        [res.results[c]["out"].reshape(SPC, COUT, H, W)
         for c in range(N_CORES)], axis=0)
    return out, res


def kernel(**inputs) -> np.ndarray:
    out, _ = run(inputs)
    return out


# revision 48
# speedup vs baseline: 1.0374x; 1.0374x over previous
"""Trainium2 Bass kernel for Conv2dWeightModulate (StyleGAN2-style modulated conv).

Math restructure 1 (modulation): the per-sample modulated conv
    out[b] = conv(conv_w * c * style[b,cin] * sigma_inv[b,cout], x_pad[b])
is rewritten as
    out[b,cout] = sigma_inv[b,cout] * conv(conv_w, (x[b] * c*style[b,cin])_pad)
so the conv weights are sample-independent (resident in SBUF) and the
per-sample modulation becomes a per-input-channel scale of x plus a
per-output-channel scale of the result. sigma has the closed form
    sigma^2[b,cout] = c^2 * sum_cin style[b,cin]^2 * sum_k conv_w[cout,cin,k]^2
computed on host (tiny [B,CIN] x [CIN,COUT] product), as is the 3-layer
mapping network producing style (all [16,512]-sized, <0.01% of FLOPs).

Math restructure 2 (Winograd F(4,3) along the height axis): each quad of
output rows (4q..4q+3) is computed from 6 transformed input rows
    v0 = 4d0-5d2+d4, v1 = -4(d1+d2)+(d3+d4), v2 = 4(d1-d2)+(d4-d3),
    v3 = 2(d3-d1)+(d4-d2), v4 = -2(d3-d1)+(d4-d2), v5 = 4d1-5d3+d5
with host-transformed weights U = G @ w over the kh axis
    (G = [[1/4,0,0],[-1/6,-1/6,-1/6],[-1/6,1/6,-1/6],
          [1/24,1/12,1/6],[1/24,-1/12,1/6],[0,0,1]])
and output rows
    out0 = m0+m1+m2+m3+m4,  out1 = m1-m2+2(m3-m4),
    out2 = (m1+m2)+4(m3+m4), out3 = m1-m2+8(m3-m4)+m5
where M[pos] = sum_{cin,kw} U[pos,kw] * v[pos] (shifted by kw).
This cuts tensor-engine MACs 2x vs direct conv (18 accumulation steps per
8 output rows instead of 36); the width axis stays direct (3 taps against
a replicate-padded 66-wide SBUF image).

Device: data-parallel over batch, 2 samples per core on 8 cores, fp16
operands (PE at 1 col/cycle, fp32 PSUM accumulate). Per (sample, chunk of
8 quads, cout-block): 6 PSUM banks hold M[0..5] for 8 quads x 64 cols; 72
accumulating 128x128 @ 128x512 matmuls fill them pos-major; VectorE forms
the four output-row combinations in fp32 (using fused scalar_tensor_tensor
for the x2/x4/x8 taps), ScalarE applies sigma_inv, and the input row
transform for the NEXT chunk is interleaved into the vector queue between
combine groups so the tensor engine never waits on it. A per-sample
power-of-2 prescale keeps scaled x in fp16's normal range and is undone
exactly in the fp32 output scale.
"""

import numpy as np
from contextlib import ExitStack

import concourse.bass as bass
import concourse.tile as tile
from concourse import bacc, mybir
from concourse import bass_utils

B, CIN, COUT, KS, H, W, DLAT = 16, 512, 512, 3, 64, 64, 512
EPS = 1e-8
N_CORES = 8
SPC = B // N_CORES          # samples per core
NCB = CIN // 128            # cin blocks
NOB = COUT // 128           # cout blocks
NPOS = 6                    # winograd F(4,3) positions
NQ = H // 4                 # row quads per sample (16)
QPC = 8                     # quads per PSUM chunk (8 quads * 64 = 512)
NCH = NQ // QPC             # chunks per sample (2)
PADW = W + 2
NROWS = 33                  # original rows staged per chunk
_cache = {}

_MUL = mybir.AluOpType.mult
_ADD = mybir.AluOpType.add


def _build():
    if "nc" in _cache:
        return _cache["nc"]
    f32 = mybir.dt.float32
    f16 = mybir.dt.float16
    nc = bacc.Bacc("TRN2", target_bir_lowering=False, debug=False,
                   num_devices=N_CORES)
    x_d = nc.dram_tensor("x", [SPC, CIN, H, W], f16, kind="ExternalInput").ap()
    # U[pos, ob, cb, cin128, kw*cout128]
    wt_d = nc.dram_tensor("wt", [NPOS, NOB, NCB, 128, KS * 128], f16,
                          kind="ExternalInput").ap()
    sig_d = nc.dram_tensor("sig", [128, SPC, NOB], f32,
                           kind="ExternalInput").ap()
    out_d = nc.dram_tensor("out", [SPC, COUT, H * W], f32,
                           kind="ExternalOutput").ap()

    with tile.TileContext(nc) as tc, ExitStack() as ctx:
        cpool = ctx.enter_context(tc.tile_pool(name="const", bufs=1))
        stpool = ctx.enter_context(tc.tile_pool(name="stage", bufs=4))
        vpool = ctx.enter_context(tc.tile_pool(name="v", bufs=2))
        vtpool = ctx.enter_context(tc.tile_pool(name="vt", bufs=8))
        ctpool = ctx.enter_context(tc.tile_pool(name="ct", bufs=8))
        opool = ctx.enter_context(tc.tile_pool(name="o", bufs=3))
        epool = ctx.enter_context(tc.tile_pool(name="e", bufs=8))
        pspool = ctx.enter_context(tc.tile_pool(name="ps", bufs=8, space="PSUM"))

        wt_sb = cpool.tile([128, NPOS, NOB, NCB, KS, 128], f16)
        sig_sb = cpool.tile([128, SPC, NOB], f32)

        # PE pre-warm: dummy matmuls bridging until the first real matmul
        # (~13us in), so the HAM clock-gate stays at 8/8 throughout
        warm_t = cpool.tile([128, QPC * W], f16)
        warm_ps = pspool.tile([128, QPC * W], f32, name="warm_ps", tag="ps")
        nc.gpsimd.memset(warm_t[:], 0.0)
        for _ in range(80):
            nc.tensor.matmul(warm_ps[:, 0:64], warm_t[:, 0:128],
                             warm_t[:, 0:64], start=True, stop=True)
        for _ in range(24):
            nc.tensor.matmul(warm_ps[:], warm_t[:, 0:128], warm_t[:],
                             start=True, stop=True)

        st_tiles = {}

        def emit_st_dmas(s, ch, cbs=range(NCB)):
            # stage the 33 original rows covering chunk ch of sample s
            r0 = 0 if ch == 0 else H - NROWS
            for cb in cbs:
                st = stpool.tile([128, NROWS * W], f16, name="st", tag="st")
                nc.sync.dma_start(
                    st[:],
                    x_d[s, cb * 128:(cb + 1) * 128,
                        r0:r0 + NROWS, :].rearrange("c a b -> c (a b)"))
                st_tiles[(s, ch, cb)] = st

        def v_tile(s, ch):
            key = ("v", s, ch)
            if key not in _cache_v:
                _cache_v[key] = vpool.tile([128, NCB, NPOS, QPC, PADW], f16,
                                           name=f"v{s}{ch}", tag="v")
            return _cache_v[key]

        _cache_v = {}

        def emit_vbuild_pos(s, ch, cb, tag):
            """Build V positions for one cin block: tag in {0,1,2,34,5}.

            d_i views: chunk0 uses d_i = st[4q+i-1] (pad row 0 duplicated,
            so only v0's q=0 needs a fixup); chunk1 uses d_i = st[4q+i]
            (pad row 65 duplicated, so only v5's q=7 needs a fixup).
            Replicate-padding commutes with the (linear) row transform, so
            V's column pads are plain copies. pos3/4 and the pads run on
            the otherwise-idle GpSimd engine to unload VectorE.
            """
            v_t = v_tile(s, ch)
            off = -1 if ch == 0 else 0
            st = st_tiles[(s, ch, cb)]
            sr = st.rearrange("c (a b) -> c a b", b=W)

            def d(i, q0=0, nq=QPC):
                lo = 4 * q0 + i + off
                return sr[:, lo:lo + 4 * (nq - 1) + 1:4]

            vi = v_t[:, cb]
            if tag == 0:
                # pos0: v0 = 4 d0 - 5 d2 + d4
                g1 = vtpool.tile([128, QPC, W], f16, name="g1", tag="vt")
                nc.vector.scalar_tensor_tensor(
                    g1[:], d(2), -5.0, d(4), _MUL, _ADD)
                if ch == 0:
                    nc.vector.scalar_tensor_tensor(
                        vi[:, 0, 1:, 1:W + 1], d(0, q0=1, nq=QPC - 1),
                        4.0, g1[:, 1:], _MUL, _ADD)
                    # q=0 fixup: d0 = clamped row = sr[0]
                    nc.vector.scalar_tensor_tensor(
                        vi[:, 0, 0:1, 1:W + 1], sr[:, 0:1], 4.0,
                        g1[:, 0:1], _MUL, _ADD)
                else:
                    nc.vector.scalar_tensor_tensor(
                        vi[:, 0, :, 1:W + 1], d(0), 4.0, g1[:], _MUL, _ADD)
            elif tag == 1:
                # pos1: v1 = -4(d1+d2) + (d3+d4)
                s12 = vtpool.tile([128, QPC, W], f16, name="s12", tag="vt")
                t34 = vtpool.tile([128, QPC, W], f16, name="t34", tag="vt")
                nc.vector.tensor_add(s12[:], d(1), d(2))
                nc.vector.tensor_add(t34[:], d(3), d(4))
                nc.vector.scalar_tensor_tensor(
                    vi[:, 1, :, 1:W + 1], s12[:], -4.0, t34[:], _MUL, _ADD)
            elif tag == 2:
                # pos2: v2 = 4(d1-d2) + (d4-d3)
                m12 = vtpool.tile([128, QPC, W], f16, name="m12", tag="vt")
                n43 = vtpool.tile([128, QPC, W], f16, name="n43", tag="vt")
                nc.vector.tensor_sub(m12[:], d(1), d(2))
                nc.vector.tensor_sub(n43[:], d(4), d(3))
                nc.vector.scalar_tensor_tensor(
                    vi[:, 2, :, 1:W + 1], m12[:], 4.0, n43[:], _MUL, _ADD)
            elif tag == 34:
                # pos3: v3 = 2(d3-d1) + (d4-d2); pos4: v4 = -2(d3-d1)+(d4-d2)
                a = vtpool.tile([128, QPC, W], f16, name="a", tag="vt")
                bb = vtpool.tile([128, QPC, W], f16, name="b", tag="vt")
                nc.vector.tensor_sub(a[:], d(3), d(1))
                nc.vector.tensor_sub(bb[:], d(4), d(2))
                nc.vector.scalar_tensor_tensor(
                    vi[:, 3, :, 1:W + 1], a[:], 2.0, bb[:], _MUL, _ADD)
                nc.vector.scalar_tensor_tensor(
                    vi[:, 4, :, 1:W + 1], a[:], -2.0, bb[:], _MUL, _ADD)
            elif tag == -34:
                # tag 34 on GpSimd (TT-only: no scalar_tensor_tensor there);
                # used for the latency-critical first chunk to parallelize
                # the transform across engines
                a = vtpool.tile([128, QPC, W], f16, name="a", tag="vt")
                bb = vtpool.tile([128, QPC, W], f16, name="b", tag="vt")
                a2 = vtpool.tile([128, QPC, W], f16, name="a2", tag="vt")
                nc.gpsimd.tensor_sub(a[:], d(3), d(1))
                nc.gpsimd.tensor_sub(bb[:], d(4), d(2))
                nc.gpsimd.tensor_add(a2[:], a[:], a[:])
                nc.gpsimd.tensor_add(vi[:, 3, :, 1:W + 1], a2[:], bb[:])
                nc.gpsimd.tensor_sub(vi[:, 4, :, 1:W + 1], bb[:], a2[:])
            elif tag == 5:
                # pos5: v5 = 4 d1 - 5 d3 + d5
                g2 = vtpool.tile([128, QPC, W], f16, name="g2", tag="vt")
                if ch == 0:
                    nc.vector.scalar_tensor_tensor(
                        g2[:], d(3), -5.0, d(5), _MUL, _ADD)
                    nc.vector.scalar_tensor_tensor(
                        vi[:, 5, :, 1:W + 1], d(1), 4.0, g2[:], _MUL, _ADD)
                else:
                    nc.vector.scalar_tensor_tensor(
                        g2[:, 0:QPC - 1], d(3, nq=QPC - 1), -5.0,
                        d(5, nq=QPC - 1), _MUL, _ADD)
                    # q=7 fixup: d5 = clamped row = sr[32]
                    nc.vector.scalar_tensor_tensor(
                        g2[:, QPC - 1:QPC], sr[:, 31:32], -5.0,
                        sr[:, 32:33], _MUL, _ADD)
                    nc.vector.scalar_tensor_tensor(
                        vi[:, 5, :, 1:W + 1], d(1), 4.0, g2[:], _MUL, _ADD)

        def emit_pads(s, ch, sl):
            # V column pads (replicate: padded col0==col1, 65==64) for the
            # given position slice, across all cin blocks, on ScalarE
            v_t = v_tile(s, ch)
            nc.scalar.copy(v_t[:, :, sl, :, 0], v_t[:, :, sl, :, 1])
            nc.scalar.copy(v_t[:, :, sl, :, W + 1], v_t[:, :, sl, :, W])

        def emit_vbuild_group(s, ch, group):
            # group 0: pos 0,1; group 1: pos 2,3,4; group 2: pos 5
            tags = ((0, 1), (2, 34), (5,))[group]
            for cb in range(NCB):
                for t in tags:
                    emit_vbuild_pos(s, ch, cb, t)
            emit_pads(s, ch, (slice(0, 2), slice(2, 5), slice(5, 6))[group])

        def emit_tileset(s, ch, ob, last=False, split_dma=False):
            v_t = v_tile(s, ch)
            pts = [pspool.tile([128, QPC * W], f32, name="ps", tag="ps")
                   for _ in range(NPOS)]
            es = [epool.tile([128, QPC * W], f32, name=f"e{i}", tag="e")
                  for i in range(NPOS)]
            ct = {}

            def combine(nm, fn):
                t = ctpool.tile([128, QPC * W], f32, name=nm, tag="ct")
                fn(t)
                ct[nm] = t

            # row-phase i of quad p is output row 4p+i: storing [p][i][w]
            # makes both sides of the output DMA fully contiguous
            out4 = opool.tile([128, QPC, 4, W], f32, name="out4", tag="o")
            outs = [out4[:, :, i, :] for i in range(4)]
            row0 = ch * QPC * 4
            dst = out_d[s, ob * 128:(ob + 1) * 128,
                        row0 * W:(row0 + 4 * QPC) * W]

            def mm(pos, cb):
                for kw in range(KS):
                    nc.tensor.matmul(
                        pts[pos][:],
                        wt_sb[:, pos, ob, cb, kw, :],
                        v_t[:, cb, pos, :, kw:kw + W],
                        start=(cb == 0 and kw == 0),
                        stop=(cb == NCB - 1 and kw == KS - 1))

            def post(pos):
                # sigma_inv-scaled PSUM eviction on ScalarE right after the
                # position's accumulation stops (frees the bank early); the
                # A^T combines then run purely in SBUF on VectorE
                nc.scalar.mul(es[pos][:], pts[pos][:], sig_sb[:, s, ob:ob + 1])
                if pos == 2:
                    # p=m1+m2, q=m1-m2, u=m0+p (all pre-scaled by sigma)
                    combine("p", lambda t: nc.vector.tensor_add(
                        t[:], es[1][:], es[2][:]))
                    combine("q", lambda t: nc.vector.tensor_sub(
                        t[:], es[1][:], es[2][:]))
                    combine("u", lambda t: nc.vector.tensor_add(
                        t[:], ct["p"][:], es[0][:]))
                elif pos == 4:
                    # r=m3+m4, t=m3-m4, then out0..2 and the out3 partial
                    combine("r", lambda t: nc.vector.tensor_add(
                        t[:], es[3][:], es[4][:]))
                    combine("t", lambda t: nc.vector.tensor_sub(
                        t[:], es[3][:], es[4][:]))
                    r3 = lambda t: t.rearrange("c (p w) -> c p w", w=W)
                    nc.vector.tensor_add(outs[0], r3(ct["u"][:]),
                                         r3(ct["r"][:]))
                    nc.vector.scalar_tensor_tensor(
                        outs[1], r3(ct["t"][:]), 2.0, r3(ct["q"][:]),
                        _MUL, _ADD)
                    nc.vector.scalar_tensor_tensor(
                        outs[2], r3(ct["r"][:]), 4.0, r3(ct["p"][:]),
                        _MUL, _ADD)
                    combine("s3", lambda t: nc.vector.scalar_tensor_tensor(
                        t[:], ct["t"][:], 8.0, ct["q"][:], _MUL, _ADD))
                    if split_dma:
                        # drain row phases 0-2 early so the final chunk's
                        # tail is not gated on one big transfer
                        d4 = dst.rearrange("c (p i w) -> c p i w", i=4, w=W)
                        for i in range(3):
                            nc.sync.dma_start(d4[:, :, i, :], outs[i])
                elif pos == 5:
                    r3 = lambda t: t.rearrange("c (p w) -> c p w", w=W)
                    nc.vector.tensor_add(outs[3], r3(ct["s3"][:]),
                                         r3(es[5][:]))
                    if split_dma:
                        d4 = dst.rearrange("c (p i w) -> c p i w", i=4, w=W)
                        nc.sync.dma_start(d4[:, :, 3, :], outs[3])
                    else:
                        # one merged, fully-contiguous DMA for all 32 rows
                        nc.sync.dma_start(
                            dst, out4.rearrange("c p i w -> c (p i w)"))

            for pos in range(NPOS):
                for cb in range(NCB):
                    mm(pos, cb)
                post(pos)

        # ob-0 weight slices land pos-major first (~2.4MB) so the first
        # tile-set never waits on the 9.4MB full weight transfer; the
        # remaining ob slices follow in bulk (one DMA per pos each way to
        # keep the Sync issue queue short)
        def emit_wt_ob0(pos):
            nc.sync.dma_start(
                wt_sb[:, pos, 0].rearrange("c b k w -> c b (k w)"),
                wt_d[pos, 0].rearrange("b c k -> c b k"))

        def emit_wt_rest(pos):
            nc.sync.dma_start(
                wt_sb[:, pos, 1:].rearrange("c o b k w -> c o b (k w)"),
                wt_d[pos, 1:].rearrange("o b c k -> c o b k"))

        chunks = [(s, ch) for s in range(SPC) for ch in range(NCH)]
        # DMA order: x block 0 and the pos-0/ob-0 weights first, so block
        # 0's transform and its pos-0 matmuls start as early as possible
        emit_st_dmas(*chunks[0], cbs=(0,))
        emit_wt_ob0(0)
        emit_st_dmas(*chunks[0], cbs=(1, 2, 3))
        nc.sync.dma_start(sig_sb[:], sig_d[:])
        for pos in range(1, NPOS):
            emit_wt_ob0(pos)
        # first chunk: pos-granular transform emission with per-cb pads so
        # the pos-0 matmuls of each cin block start as soon as that block
        # is built (matmul waits are per-instruction in the tensor queue);
        # pos 3/4 of blocks 2-3 run on GpSimd to halve the serial latency
        s0, c0 = chunks[0]
        v_t0 = v_tile(s0, c0)
        for ti, tag in enumerate((0, 1, 2, 34, 5)):
            sl = (slice(0, 1), slice(1, 2), slice(2, 3),
                  slice(3, 5), slice(5, 6))[ti]
            for cb in range(NCB):
                emit_vbuild_pos(s0, c0, cb, tag)
                nc.scalar.copy(v_t0[:, cb, sl, :, 0], v_t0[:, cb, sl, :, 1])
                nc.scalar.copy(v_t0[:, cb, sl, :, W + 1],
                               v_t0[:, cb, sl, :, W])
            if ti < 3:
                emit_wt_rest(2 * ti)
                emit_wt_rest(2 * ti + 1)
        for ci, (s, ch) in enumerate(chunks):
            for ob in range(NOB):
                tl = ci == len(chunks) - 1
                emit_tileset(s, ch, ob, last=tl,
                             split_dma=(tl and ob == NOB - 1))
                # interleave the next chunk's input transform into the
                # vector queue so it overlaps this chunk's matmuls
                if ci + 1 < len(chunks) and ob < 3:
                    if ob == 0:
                        emit_st_dmas(*chunks[ci + 1])
                    emit_vbuild_group(*chunks[ci + 1], ob)
    nc.compile()
    _cache["nc"] = nc
    return nc


def _prelu(z, a):
    return np.where(z >= 0, z, a * z)


_G = np.array([[1 / 4, 0, 0],
               [-1 / 6, -1 / 6, -1 / 6],
               [-1 / 6, 1 / 6, -1 / 6],
               [1 / 24, 1 / 12, 1 / 6],
               [1 / 24, -1 / 12, 1 / 6],
               [0, 0, 1]], dtype=np.float64)


def _prepare(inputs):
    x = np.asarray(inputs["x"], dtype=np.float32)
    s = np.asarray(inputs["s"], dtype=np.float32)
    map_w0 = np.asarray(inputs["map_w0"], dtype=np.float32)
    map_b0 = np.asarray(inputs["map_b0"], dtype=np.float32)
    a0 = np.asarray(inputs["prelu_a0"], dtype=np.float32)
    map_w1 = np.asarray(inputs["map_w1"], dtype=np.float32)
    map_b1 = np.asarray(inputs["map_b1"], dtype=np.float32)
    a1 = np.asarray(inputs["prelu_a1"], dtype=np.float32)
    style_w = np.asarray(inputs["style_w"], dtype=np.float32)
    style_b = np.asarray(inputs["style_b"], dtype=np.float32)
    conv_w = np.asarray(inputs["conv_w"], dtype=np.float32)

    c_lin = np.float32(1.0 / np.sqrt(DLAT))
    z = _prelu(s @ (map_w0 * c_lin).T + map_b0, a0)
    z = _prelu(z @ (map_w1 * c_lin).T + map_b1, a1)
    style = z @ (style_w * c_lin).T + style_b          # [B, CIN]

    c_conv = 1.0 / np.sqrt(CIN * KS * KS)
    w2 = ((conv_w.astype(np.float64) * c_conv) ** 2).sum(axis=(2, 3))  # [COUT, CIN]
    sig2 = (style.astype(np.float64) ** 2) @ w2.T                      # [B, COUT]
    sig_inv = (1.0 / np.sqrt(sig2 + EPS)).astype(np.float32)
    msc = (style * np.float32(c_conv)).astype(np.float32)              # [B, CIN]

    # per-sample power-of-2 normalizer keeps msc*x in fp16's normal range;
    # undone exactly in the fp32 output scale
    rms = np.sqrt(np.mean((msc.astype(np.float64)) ** 2, axis=1)) + 1e-30
    k = np.clip(np.round(-np.log2(rms)), -20, 40).astype(np.int32)     # [B]
    pw = np.exp2(k.astype(np.float32))                                  # 2^k
    msc_n = msc * pw[:, None]
    sig_n = sig_inv / pw[:, None]

    # fold the per-cin style scale into x on host, cast fp16
    x_scaled = (x * msc_n[:, :, None, None]).astype(np.float16)

    # winograd F(4,3) weight transform over kh: U[pos] = sum_kh G[pos,kh] w
    # conv_w: [COUT, CIN, KH, KW] -> U: [NPOS, NOB, NCB, 128cin, KW*128cout]
    u = np.einsum("pk,ockw->ocpw", _G, conv_w.astype(np.float64)).astype(np.float16)
    wt_host = np.ascontiguousarray(
        u.reshape(NOB, 128, NCB, 128, NPOS, KS).transpose(4, 0, 2, 3, 5, 1)
        .reshape(NPOS, NOB, NCB, 128, KS * 128))

    sig_r = sig_n.reshape(B, NOB, 128)
    in_maps = []
    for c in range(N_CORES):
        sl = slice(c * SPC, (c + 1) * SPC)
        in_maps.append({
            "x": np.ascontiguousarray(x_scaled[sl]),
            "wt": wt_host,
            "sig": np.ascontiguousarray(sig_r[sl].transpose(2, 0, 1)),
        })
    return in_maps


def run(inputs, **spmd_kwargs):
    nc = _build()
    in_maps = _prepare(inputs)
    res = bass_utils.run_bass_kernel_spmd(
        nc, in_maps, core_ids=list(range(N_CORES)), **spmd_kwargs)
    out = np.concatenate(
        [res.results[c]["out"].reshape(SPC, COUT, H, W)
         for c in range(N_CORES)], axis=0)
    return out, res


def kernel(**inputs) -> np.ndarray:
    out, _ = run(inputs)
    return out


# revision 49
# speedup vs baseline: 1.0421x; 1.0045x over previous
"""Trainium2 Bass kernel for Conv2dWeightModulate (StyleGAN2-style modulated conv).

Math restructure 1 (modulation): the per-sample modulated conv
    out[b] = conv(conv_w * c * style[b,cin] * sigma_inv[b,cout], x_pad[b])
is rewritten as
    out[b,cout] = sigma_inv[b,cout] * conv(conv_w, (x[b] * c*style[b,cin])_pad)
so the conv weights are sample-independent (resident in SBUF) and the
per-sample modulation becomes a per-input-channel scale of x plus a
per-output-channel scale of the result. sigma has the closed form
    sigma^2[b,cout] = c^2 * sum_cin style[b,cin]^2 * sum_k conv_w[cout,cin,k]^2
computed on host (tiny [B,CIN] x [CIN,COUT] product), as is the 3-layer
mapping network producing style (all [16,512]-sized, <0.01% of FLOPs).

Math restructure 2 (Winograd F(4,3) along the height axis): each quad of
output rows (4q..4q+3) is computed from 6 transformed input rows
    v0 = 4d0-5d2+d4, v1 = -4(d1+d2)+(d3+d4), v2 = 4(d1-d2)+(d4-d3),
    v3 = 2(d3-d1)+(d4-d2), v4 = -2(d3-d1)+(d4-d2), v5 = 4d1-5d3+d5
with host-transformed weights U = G @ w over the kh axis
    (G = [[1/4,0,0],[-1/6,-1/6,-1/6],[-1/6,1/6,-1/6],
          [1/24,1/12,1/6],[1/24,-1/12,1/6],[0,0,1]])
and output rows
    out0 = m0+m1+m2+m3+m4,  out1 = m1-m2+2(m3-m4),
    out2 = (m1+m2)+4(m3+m4), out3 = m1-m2+8(m3-m4)+m5
where M[pos] = sum_{cin,kw} U[pos,kw] * v[pos] (shifted by kw).
This cuts tensor-engine MACs 2x vs direct conv (18 accumulation steps per
8 output rows instead of 36); the width axis stays direct (3 taps against
a replicate-padded 66-wide SBUF image).

Device: data-parallel over batch, 2 samples per core on 8 cores, fp16
operands (PE at 1 col/cycle, fp32 PSUM accumulate). Per (sample, chunk of
8 quads, cout-block): 6 PSUM banks hold M[0..5] for 8 quads x 64 cols; 72
accumulating 128x128 @ 128x512 matmuls fill them pos-major; VectorE forms
the four output-row combinations in fp32 (using fused scalar_tensor_tensor
for the x2/x4/x8 taps), ScalarE applies sigma_inv, and the input row
transform for the NEXT chunk is interleaved into the vector queue between
combine groups so the tensor engine never waits on it. A per-sample
power-of-2 prescale keeps scaled x in fp16's normal range and is undone
exactly in the fp32 output scale.
"""

import numpy as np
from contextlib import ExitStack

import concourse.bass as bass
import concourse.tile as tile
from concourse import bacc, mybir
from concourse import bass_utils

B, CIN, COUT, KS, H, W, DLAT = 16, 512, 512, 3, 64, 64, 512
EPS = 1e-8
N_CORES = 8
SPC = B // N_CORES          # samples per core
NCB = CIN // 128            # cin blocks
NOB = COUT // 128           # cout blocks
NPOS = 6                    # winograd F(4,3) positions
NQ = H // 4                 # row quads per sample (16)
QPC = 8                     # quads per PSUM chunk (8 quads * 64 = 512)
NCH = NQ // QPC             # chunks per sample (2)
PADW = W + 2
NROWS = 33                  # original rows staged per chunk
_cache = {}

_MUL = mybir.AluOpType.mult
_ADD = mybir.AluOpType.add


def _build():
    if "nc" in _cache:
        return _cache["nc"]
    f32 = mybir.dt.float32
    f16 = mybir.dt.float16
    nc = bacc.Bacc("TRN2", target_bir_lowering=False, debug=False,
                   num_devices=N_CORES)
    x_d = nc.dram_tensor("x", [SPC, CIN, H, W], f16, kind="ExternalInput").ap()
    # U[pos, ob, cb, cin128, kw*cout128]
    wt_d = nc.dram_tensor("wt", [NPOS, NOB, NCB, 128, KS * 128], f16,
                          kind="ExternalInput").ap()
    sig_d = nc.dram_tensor("sig", [128, SPC, NOB], f32,
                           kind="ExternalInput").ap()
    out_d = nc.dram_tensor("out", [SPC, COUT, H * W], f32,
                           kind="ExternalOutput").ap()

    with tile.TileContext(nc) as tc, ExitStack() as ctx:
        cpool = ctx.enter_context(tc.tile_pool(name="const", bufs=1))
        stpool = ctx.enter_context(tc.tile_pool(name="stage", bufs=4))
        vpool = ctx.enter_context(tc.tile_pool(name="v", bufs=2))
        vtpool = ctx.enter_context(tc.tile_pool(name="vt", bufs=8))
        ctpool = ctx.enter_context(tc.tile_pool(name="ct", bufs=8))
        opool = ctx.enter_context(tc.tile_pool(name="o", bufs=3))
        epool = ctx.enter_context(tc.tile_pool(name="e", bufs=8))
        pspool = ctx.enter_context(tc.tile_pool(name="ps", bufs=8, space="PSUM"))

        wt_sb = cpool.tile([128, NPOS, NOB, NCB, KS, 128], f16)
        sig_sb = cpool.tile([128, SPC, NOB], f32)

        # PE pre-warm: dummy matmuls bridging until the first real matmul
        # (~13us in), so the HAM clock-gate stays at 8/8 throughout
        warm_t = cpool.tile([128, QPC * W], f16)
        warm_ps = pspool.tile([128, QPC * W], f32, name="warm_ps", tag="ps")
        nc.gpsimd.memset(warm_t[:], 0.0)
        for _ in range(80):
            nc.tensor.matmul(warm_ps[:, 0:64], warm_t[:, 0:128],
                             warm_t[:, 0:64], start=True, stop=True)
        for _ in range(8):
            nc.tensor.matmul(warm_ps[:], warm_t[:, 0:128], warm_t[:],
                             start=True, stop=True)

        st_tiles = {}

        def emit_st_dmas(s, ch, cbs=range(NCB)):
            # stage the 33 original rows covering chunk ch of sample s
            r0 = 0 if ch == 0 else H - NROWS
            for cb in cbs:
                st = stpool.tile([128, NROWS * W], f16, name="st", tag="st")
                nc.sync.dma_start(
                    st[:],
                    x_d[s, cb * 128:(cb + 1) * 128,
                        r0:r0 + NROWS, :].rearrange("c a b -> c (a b)"))
                st_tiles[(s, ch, cb)] = st

        def v_tile(s, ch):
            key = ("v", s, ch)
            if key not in _cache_v:
                _cache_v[key] = vpool.tile([128, NCB, NPOS, QPC, PADW], f16,
                                           name=f"v{s}{ch}", tag="v")
            return _cache_v[key]

        _cache_v = {}

        def emit_vbuild_pos(s, ch, cb, tag):
            """Build V positions for one cin block: tag in {0,1,2,34,5}.

            d_i views: chunk0 uses d_i = st[4q+i-1] (pad row 0 duplicated,
            so only v0's q=0 needs a fixup); chunk1 uses d_i = st[4q+i]
            (pad row 65 duplicated, so only v5's q=7 needs a fixup).
            Replicate-padding commutes with the (linear) row transform, so
            V's column pads are plain copies. pos3/4 and the pads run on
            the otherwise-idle GpSimd engine to unload VectorE.
            """
            v_t = v_tile(s, ch)
            off = -1 if ch == 0 else 0
            st = st_tiles[(s, ch, cb)]
            sr = st.rearrange("c (a b) -> c a b", b=W)

            def d(i, q0=0, nq=QPC):
                lo = 4 * q0 + i + off
                return sr[:, lo:lo + 4 * (nq - 1) + 1:4]

            vi = v_t[:, cb]
            if tag == 0:
                # pos0: v0 = 4 d0 - 5 d2 + d4
                g1 = vtpool.tile([128, QPC, W], f16, name="g1", tag="vt")
                nc.vector.scalar_tensor_tensor(
                    g1[:], d(2), -5.0, d(4), _MUL, _ADD)
                if ch == 0:
                    nc.vector.scalar_tensor_tensor(
                        vi[:, 0, 1:, 1:W + 1], d(0, q0=1, nq=QPC - 1),
                        4.0, g1[:, 1:], _MUL, _ADD)
                    # q=0 fixup: d0 = clamped row = sr[0]
                    nc.vector.scalar_tensor_tensor(
                        vi[:, 0, 0:1, 1:W + 1], sr[:, 0:1], 4.0,
                        g1[:, 0:1], _MUL, _ADD)
                else:
                    nc.vector.scalar_tensor_tensor(
                        vi[:, 0, :, 1:W + 1], d(0), 4.0, g1[:], _MUL, _ADD)
            elif tag == 1:
                # pos1: v1 = -4(d1+d2) + (d3+d4)
                s12 = vtpool.tile([128, QPC, W], f16, name="s12", tag="vt")
                t34 = vtpool.tile([128, QPC, W], f16, name="t34", tag="vt")
                nc.vector.tensor_add(s12[:], d(1), d(2))
                nc.vector.tensor_add(t34[:], d(3), d(4))
                nc.vector.scalar_tensor_tensor(
                    vi[:, 1, :, 1:W + 1], s12[:], -4.0, t34[:], _MUL, _ADD)
            elif tag == 2:
                # pos2: v2 = 4(d1-d2) + (d4-d3)
                m12 = vtpool.tile([128, QPC, W], f16, name="m12", tag="vt")
                n43 = vtpool.tile([128, QPC, W], f16, name="n43", tag="vt")
                nc.vector.tensor_sub(m12[:], d(1), d(2))
                nc.vector.tensor_sub(n43[:], d(4), d(3))
                nc.vector.scalar_tensor_tensor(
                    vi[:, 2, :, 1:W + 1], m12[:], 4.0, n43[:], _MUL, _ADD)
            elif tag == 34:
                # pos3: v3 = 2(d3-d1) + (d4-d2); pos4: v4 = -2(d3-d1)+(d4-d2)
                a = vtpool.tile([128, QPC, W], f16, name="a", tag="vt")
                bb = vtpool.tile([128, QPC, W], f16, name="b", tag="vt")
                nc.vector.tensor_sub(a[:], d(3), d(1))
                nc.vector.tensor_sub(bb[:], d(4), d(2))
                nc.vector.scalar_tensor_tensor(
                    vi[:, 3, :, 1:W + 1], a[:], 2.0, bb[:], _MUL, _ADD)
                nc.vector.scalar_tensor_tensor(
                    vi[:, 4, :, 1:W + 1], a[:], -2.0, bb[:], _MUL, _ADD)
            elif tag == -34:
                # tag 34 on GpSimd (TT-only: no scalar_tensor_tensor there);
                # used for the latency-critical first chunk to parallelize
                # the transform across engines
                a = vtpool.tile([128, QPC, W], f16, name="a", tag="vt")
                bb = vtpool.tile([128, QPC, W], f16, name="b", tag="vt")
                a2 = vtpool.tile([128, QPC, W], f16, name="a2", tag="vt")
                nc.gpsimd.tensor_sub(a[:], d(3), d(1))
                nc.gpsimd.tensor_sub(bb[:], d(4), d(2))
                nc.gpsimd.tensor_add(a2[:], a[:], a[:])
                nc.gpsimd.tensor_add(vi[:, 3, :, 1:W + 1], a2[:], bb[:])
                nc.gpsimd.tensor_sub(vi[:, 4, :, 1:W + 1], bb[:], a2[:])
            elif tag == 5:
                # pos5: v5 = 4 d1 - 5 d3 + d5
                g2 = vtpool.tile([128, QPC, W], f16, name="g2", tag="vt")
                if ch == 0:
                    nc.vector.scalar_tensor_tensor(
                        g2[:], d(3), -5.0, d(5), _MUL, _ADD)
                    nc.vector.scalar_tensor_tensor(
                        vi[:, 5, :, 1:W + 1], d(1), 4.0, g2[:], _MUL, _ADD)
                else:
                    nc.vector.scalar_tensor_tensor(
                        g2[:, 0:QPC - 1], d(3, nq=QPC - 1), -5.0,
                        d(5, nq=QPC - 1), _MUL, _ADD)
                    # q=7 fixup: d5 = clamped row = sr[32]
                    nc.vector.scalar_tensor_tensor(
                        g2[:, QPC - 1:QPC], sr[:, 31:32], -5.0,
                        sr[:, 32:33], _MUL, _ADD)
                    nc.vector.scalar_tensor_tensor(
                        vi[:, 5, :, 1:W + 1], d(1), 4.0, g2[:], _MUL, _ADD)

        def emit_pads(s, ch, sl):
            # V column pads (replicate: padded col0==col1, 65==64) for the
            # given position slice, across all cin blocks, on ScalarE
            v_t = v_tile(s, ch)
            nc.scalar.copy(v_t[:, :, sl, :, 0], v_t[:, :, sl, :, 1])
            nc.scalar.copy(v_t[:, :, sl, :, W + 1], v_t[:, :, sl, :, W])

        def emit_vbuild_group(s, ch, group):
            # group 0: pos 0,1; group 1: pos 2,3,4; group 2: pos 5
            tags = ((0, 1), (2, 34), (5,))[group]
            for cb in range(NCB):
                for t in tags:
                    emit_vbuild_pos(s, ch, cb, t)
            emit_pads(s, ch, (slice(0, 2), slice(2, 5), slice(5, 6))[group])

        def emit_tileset(s, ch, ob, last=False, split_dma=False):
            v_t = v_tile(s, ch)
            pts = [pspool.tile([128, QPC * W], f32, name="ps", tag="ps")
                   for _ in range(NPOS)]
            es = [epool.tile([128, QPC * W], f32, name=f"e{i}", tag="e")
                  for i in range(NPOS)]
            ct = {}

            def combine(nm, fn):
                t = ctpool.tile([128, QPC * W], f32, name=nm, tag="ct")
                fn(t)
                ct[nm] = t

            # row-phase i of quad p is output row 4p+i: storing [p][i][w]
            # makes both sides of the output DMA fully contiguous
            out4 = opool.tile([128, QPC, 4, W], f32, name="out4", tag="o")
            outs = [out4[:, :, i, :] for i in range(4)]
            row0 = ch * QPC * 4
            dst = out_d[s, ob * 128:(ob + 1) * 128,
                        row0 * W:(row0 + 4 * QPC) * W]

            def mm(pos, cb):
                for kw in range(KS):
                    nc.tensor.matmul(
                        pts[pos][:],
                        wt_sb[:, pos, ob, cb, kw, :],
                        v_t[:, cb, pos, :, kw:kw + W],
                        start=(cb == 0 and kw == 0),
                        stop=(cb == NCB - 1 and kw == KS - 1))

            def post(pos):
                # sigma_inv-scaled PSUM eviction on ScalarE right after the
                # position's accumulation stops (frees the bank early); the
                # A^T combines then run purely in SBUF on VectorE
                nc.scalar.mul(es[pos][:], pts[pos][:], sig_sb[:, s, ob:ob + 1])
                if pos == 2:
                    # p=m1+m2, q=m1-m2, u=m0+p (all pre-scaled by sigma)
                    combine("p", lambda t: nc.vector.tensor_add(
                        t[:], es[1][:], es[2][:]))
                    combine("q", lambda t: nc.vector.tensor_sub(
                        t[:], es[1][:], es[2][:]))
                    combine("u", lambda t: nc.vector.tensor_add(
                        t[:], ct["p"][:], es[0][:]))
                elif pos == 4:
                    # r=m3+m4, t=m3-m4, then out0..2 and the out3 partial
                    combine("r", lambda t: nc.vector.tensor_add(
                        t[:], es[3][:], es[4][:]))
                    combine("t", lambda t: nc.vector.tensor_sub(
                        t[:], es[3][:], es[4][:]))
                    r3 = lambda t: t.rearrange("c (p w) -> c p w", w=W)
                    nc.vector.tensor_add(outs[0], r3(ct["u"][:]),
                                         r3(ct["r"][:]))
                    nc.vector.scalar_tensor_tensor(
                        outs[1], r3(ct["t"][:]), 2.0, r3(ct["q"][:]),
                        _MUL, _ADD)
                    nc.vector.scalar_tensor_tensor(
                        outs[2], r3(ct["r"][:]), 4.0, r3(ct["p"][:]),
                        _MUL, _ADD)
                    combine("s3", lambda t: nc.vector.scalar_tensor_tensor(
                        t[:], ct["t"][:], 8.0, ct["q"][:], _MUL, _ADD))
                    if split_dma:
                        # drain row phases 0-2 early so the final chunk's
                        # tail is not gated on one big transfer
                        d4 = dst.rearrange("c (p i w) -> c p i w", i=4, w=W)
                        for i in range(3):
                            nc.sync.dma_start(d4[:, :, i, :], outs[i])
                elif pos == 5:
                    r3 = lambda t: t.rearrange("c (p w) -> c p w", w=W)
                    nc.vector.tensor_add(outs[3], r3(ct["s3"][:]),
                                         r3(es[5][:]))
                    if split_dma:
                        d4 = dst.rearrange("c (p i w) -> c p i w", i=4, w=W)
                        nc.sync.dma_start(d4[:, :, 3, :], outs[3])
                    else:
                        # one merged, fully-contiguous DMA for all 32 rows
                        nc.sync.dma_start(
                            dst, out4.rearrange("c p i w -> c (p i w)"))

            for pos in range(NPOS):
                for cb in range(NCB):
                    mm(pos, cb)
                post(pos)

        # ob-0 weight slices land pos-major first (~2.4MB) so the first
        # tile-set never waits on the 9.4MB full weight transfer; the
        # remaining ob slices follow in bulk (one DMA per pos each way to
        # keep the Sync issue queue short)
        def emit_wt_ob0(pos):
            nc.sync.dma_start(
                wt_sb[:, pos, 0].rearrange("c b k w -> c b (k w)"),
                wt_d[pos, 0].rearrange("b c k -> c b k"))

        def emit_wt_rest(pos):
            nc.sync.dma_start(
                wt_sb[:, pos, 1:].rearrange("c o b k w -> c o b (k w)"),
                wt_d[pos, 1:].rearrange("o b c k -> c o b k"))

        chunks = [(s, ch) for s in range(SPC) for ch in range(NCH)]
        # DMA order: x block 0 and the pos-0/ob-0 weights first, so block
        # 0's transform and its pos-0 matmuls start as early as possible
        emit_st_dmas(*chunks[0], cbs=(0,))
        emit_wt_ob0(0)
        emit_st_dmas(*chunks[0], cbs=(1, 2, 3))
        nc.sync.dma_start(sig_sb[:], sig_d[:])
        for pos in range(1, NPOS):
            emit_wt_ob0(pos)
        # first chunk: pos-granular transform emission with per-cb pads so
        # the pos-0 matmuls of each cin block start as soon as that block
        # is built (matmul waits are per-instruction in the tensor queue);
        # pos 3/4 of blocks 2-3 run on GpSimd to halve the serial latency
        s0, c0 = chunks[0]
        v_t0 = v_tile(s0, c0)
        for ti, tag in enumerate((0, 1, 2, 34, 5)):
            sl = (slice(0, 1), slice(1, 2), slice(2, 3),
                  slice(3, 5), slice(5, 6))[ti]
            for cb in range(NCB):
                emit_vbuild_pos(s0, c0, cb, tag)
                nc.scalar.copy(v_t0[:, cb, sl, :, 0], v_t0[:, cb, sl, :, 1])
                nc.scalar.copy(v_t0[:, cb, sl, :, W + 1],
                               v_t0[:, cb, sl, :, W])
            if ti < 3:
                emit_wt_rest(2 * ti)
                emit_wt_rest(2 * ti + 1)
        for ci, (s, ch) in enumerate(chunks):
            for ob in range(NOB):
                tl = ci == len(chunks) - 1
                emit_tileset(s, ch, ob, last=tl,
                             split_dma=(tl and ob == NOB - 1))
                # interleave the next chunk's input transform into the
                # vector queue so it overlaps this chunk's matmuls
                if ci + 1 < len(chunks) and ob < 3:
                    if ob == 0:
                        emit_st_dmas(*chunks[ci + 1])
                    emit_vbuild_group(*chunks[ci + 1], ob)
    nc.compile()
    _cache["nc"] = nc
    return nc


def _prelu(z, a):
    return np.where(z >= 0, z, a * z)


_G = np.array([[1 / 4, 0, 0],
               [-1 / 6, -1 / 6, -1 / 6],
               [-1 / 6, 1 / 6, -1 / 6],
               [1 / 24, 1 / 12, 1 / 6],
               [1 / 24, -1 / 12, 1 / 6],
               [0, 0, 1]], dtype=np.float64)


def _prepare(inputs):
    x = np.asarray(inputs["x"], dtype=np.float32)
    s = np.asarray(inputs["s"], dtype=np.float32)
    map_w0 = np.asarray(inputs["map_w0"], dtype=np.float32)
    map_b0 = np.asarray(inputs["map_b0"], dtype=np.float32)
    a0 = np.asarray(inputs["prelu_a0"], dtype=np.float32)
    map_w1 = np.asarray(inputs["map_w1"], dtype=np.float32)
    map_b1 = np.asarray(inputs["map_b1"], dtype=np.float32)
    a1 = np.asarray(inputs["prelu_a1"], dtype=np.float32)
    style_w = np.asarray(inputs["style_w"], dtype=np.float32)
    style_b = np.asarray(inputs["style_b"], dtype=np.float32)
    conv_w = np.asarray(inputs["conv_w"], dtype=np.float32)

    c_lin = np.float32(1.0 / np.sqrt(DLAT))
    z = _prelu(s @ (map_w0 * c_lin).T + map_b0, a0)
    z = _prelu(z @ (map_w1 * c_lin).T + map_b1, a1)
    style = z @ (style_w * c_lin).T + style_b          # [B, CIN]

    c_conv = 1.0 / np.sqrt(CIN * KS * KS)
    w2 = ((conv_w.astype(np.float64) * c_conv) ** 2).sum(axis=(2, 3))  # [COUT, CIN]
    sig2 = (style.astype(np.float64) ** 2) @ w2.T                      # [B, COUT]
    sig_inv = (1.0 / np.sqrt(sig2 + EPS)).astype(np.float32)
    msc = (style * np.float32(c_conv)).astype(np.float32)              # [B, CIN]

    # per-sample power-of-2 normalizer keeps msc*x in fp16's normal range;
    # undone exactly in the fp32 output scale
    rms = np.sqrt(np.mean((msc.astype(np.float64)) ** 2, axis=1)) + 1e-30
    k = np.clip(np.round(-np.log2(rms)), -20, 40).astype(np.int32)     # [B]
    pw = np.exp2(k.astype(np.float32))                                  # 2^k
    msc_n = msc * pw[:, None]
    sig_n = sig_inv / pw[:, None]

    # fold the per-cin style scale into x on host, cast fp16
    x_scaled = (x * msc_n[:, :, None, None]).astype(np.float16)

    # winograd F(4,3) weight transform over kh: U[pos] = sum_kh G[pos,kh] w
    # conv_w: [COUT, CIN, KH, KW] -> U: [NPOS, NOB, NCB, 128cin, KW*128cout]
    u = np.einsum("pk,ockw->ocpw", _G, conv_w.astype(np.float64)).astype(np.float16)
    wt_host = np.ascontiguousarray(
        u.reshape(NOB, 128, NCB, 128, NPOS, KS).transpose(4, 0, 2, 3, 5, 1)
        .reshape(NPOS, NOB, NCB, 128, KS * 128))

    sig_r = sig_n.reshape(B, NOB, 128)
    in_maps = []
    for c in range(N_CORES):
        sl = slice(c * SPC, (c + 1) * SPC)
        in_maps.append({
            "x": np.ascontiguousarray(x_scaled[sl]),
            "wt": wt_host,
            "sig": np.ascontiguousarray(sig_r[sl].transpose(2, 0, 1)),
        })
    return in_maps


def run(inputs, **spmd_kwargs):
    nc = _build()
    in_maps = _prepare(inputs)
    res = bass_utils.run_bass_kernel_spmd(
        nc, in_maps, core_ids=list(range(N_CORES)), **spmd_kwargs)
    out = np.concatenate(
        [res.results[c]["out"].reshape(SPC, COUT, H, W)
         for c in range(N_CORES)], axis=0)
    return out, res


def kernel(**inputs) -> np.ndarray:
    out, _ = run(inputs)
    return out
